# revision 8
# baseline (speedup 1.0000x reference)
"""Trainium2 Bass kernel for a 4-layer GCN (GCNConv + LayerNorm + GELU + residual).

Strategy (8 NeuronCores):
  - Shard nodes across 8 cores: 6250 nodes/core, padded to 6272 = 49*128.
  - Edges (incl. self-loops, w=1) partitioned by destination shard, sorted by dst.
  - Per layer, per core:
      Phase A: z = LayerNorm(h_local); zs = dinv * (z @ W'_i)  (bf16), W' = diag(ln_w)@conv_w
      AllGather zs -> zs_full (bf16, all 50176 padded rows)
      Phase B: per 128-dst tile, for each 128-edge chunk: indirect-DMA gather
               src rows of zs_full, build one-hot*w selection matrix on DVE,
               matmul-accumulate into PSUM.  agg = dinv_dst * psum (+ bias terms
               folded in as K=1 matmuls); h = GELU(agg) + h.
  - Final: out = h @ post_w + post_b, computed per tile in f32.

Host precomputes: deg/dinv, edge partition+sort, padded per-tile chunk arrays
(gather row ids, weights, local dst), per-node bias coefficients.
"""

import sys

sys.path.insert(0, "/opt/trn_rl_repo")

import numpy as np

import concourse.tile as tile
from concourse import bacc, bass, mybir
from concourse.bass_utils import run_bass_kernel_spmd
from concourse.masks import make_identity

N = 50000
D = 256
L = 4
NC = 8
NS = N // NC          # 6250 real nodes per core
TILES = 49            # ceil(6250/128)
NSP = TILES * 128     # 6272 padded nodes per core
NROWS = NC * NSP      # 50176 padded rows in allgathered buffer
EPS = 1e-5

F32 = mybir.dt.float32
BF16 = mybir.dt.bfloat16
I32 = mybir.dt.int32
NP_BF16 = mybir.dt.np(mybir.dt.bfloat16)

_CACHE = {}


def build_program(nchunk: int):
    nc = bacc.Bacc()

    # ---- I/O ----
    h0 = nc.declare_dram_parameter("h0", [NSP, D], F32, isOutput=False)
    gidx = nc.declare_dram_parameter("gidx", [TILES, 128, nchunk], I32, isOutput=False)
    wv = nc.declare_dram_parameter("wv", [TILES, 128, nchunk], F32, isOutput=False)
    dstl = nc.declare_dram_parameter("dstl", [TILES, 128, nchunk], F32, isOutput=False)
    dinvc = nc.declare_dram_parameter("dinvc", [128, TILES], F32, isOutput=False)
    a1r = nc.declare_dram_parameter("a1r", [1, NSP], F32, isOutput=False)
    a2r = nc.declare_dram_parameter("a2r", [1, NSP], F32, isOutput=False)
    wp = nc.declare_dram_parameter("wp", [L, 2, 128, D], BF16, isOutput=False)
    bpp = nc.declare_dram_parameter("bpp", [1, L * D], F32, isOutput=False)
    convb = nc.declare_dram_parameter("convb", [1, L * D], F32, isOutput=False)
    postw = nc.declare_dram_parameter("postw", [2, 128, D], F32, isOutput=False)
    postb = nc.declare_dram_parameter("postb", [1, D], F32, isOutput=False)
    out = nc.declare_dram_parameter("out", [NSP, D], F32, isOutput=True)

    # ---- internal DRAM ----
    h_a = nc.dram_tensor("h_a", [NSP, D], F32)
    h_b = nc.dram_tensor("h_b", [NSP, D], F32)
    zs_loc = [nc.dram_tensor(f"zs_loc{i}", [NSP, D], BF16) for i in range(L)]
    zs_full = [
        nc.dram_tensor(f"zs_full{i}", [NROWS, D], BF16, addr_space="Shared")
        for i in range(L)
    ]

    h_src = [h0, h_b, h_a, h_b]
    h_dst = [h_b, h_a, h_b, h_a]

    with tile.TileContext(nc) as tc:
        with (
            tc.tile_pool(name="const", bufs=1) as cpool,
            tc.tile_pool(name="hin", bufs=4) as hpool,
            tc.tile_pool(name="stat", bufs=4) as spool,
            tc.tile_pool(name="zn", bufs=3) as znpool,
            tc.tile_pool(name="znt", bufs=3) as zntpool,
            tc.tile_pool(name="zs", bufs=3) as zspool,
            tc.tile_pool(name="meta", bufs=3) as mpool,
            tc.tile_pool(name="msg", bufs=8) as msgpool,
            tc.tile_pool(name="selm", bufs=8) as selpool,
            tc.tile_pool(name="epi", bufs=3) as epool,
            tc.tile_pool(name="pt", bufs=2, space="PSUM") as pt_pool,
            tc.tile_pool(name="pzw", bufs=2, space="PSUM") as pzw_pool,
            tc.tile_pool(name="pagg", bufs=2, space="PSUM") as pagg_pool,
        ):
            # ---- constants resident in SBUF ----
            ident_bf = cpool.tile([128, 128], BF16, tag="ident_bf")
            make_identity(nc, ident_bf[:])
            ident_f32 = cpool.tile([128, 128], F32, tag="ident_f32")
            make_identity(nc, ident_f32[:])

            iota_i = cpool.tile([128, 128], I32, tag="iota_i")
            nc.gpsimd.iota(iota_i[:], pattern=[[1, 128]], base=0, channel_multiplier=0)
            iota_bf = cpool.tile([128, 128], BF16, tag="iota_bf")
            nc.vector.tensor_copy(iota_bf[:], iota_i[:])

            wp_t = {}
            for i in range(L):
                for hlf in range(2):
                    t = cpool.tile([128, D], BF16, tag=f"wp{i}{hlf}")
                    nc.sync.dma_start(out=t[:], in_=wp[i, hlf])
                    wp_t[(i, hlf)] = t
            postw_t = []
            for hlf in range(2):
                t = cpool.tile([128, D], F32, tag=f"postw{hlf}")
                nc.sync.dma_start(out=t[:], in_=postw[hlf])
                postw_t.append(t)

            bpp_t = cpool.tile([1, L * D], F32, tag="bpp")
            nc.sync.dma_start(out=bpp_t[:], in_=bpp[:])
            convb_t = cpool.tile([1, L * D], F32, tag="convb")
            nc.sync.dma_start(out=convb_t[:], in_=convb[:])
            postb_t = cpool.tile([1, D], F32, tag="postb")
            nc.sync.dma_start(out=postb_t[:], in_=postb[:])
            ones_row = cpool.tile([1, 128], F32, tag="ones_row")
            nc.vector.memset(ones_row[:], 1.0)
            eps_col = cpool.tile([128, 1], F32, tag="eps_col")
            nc.vector.memset(eps_col[:], EPS)

            dinv_t = cpool.tile([128, TILES], F32, tag="dinv_t")
            nc.sync.dma_start(out=dinv_t[:], in_=dinvc[:])
            a1_t = cpool.tile([1, NSP], F32, tag="a1_t")
            nc.sync.dma_start(out=a1_t[:], in_=a1r[:])
            a2_t = cpool.tile([1, NSP], F32, tag="a2_t")
            nc.sync.dma_start(out=a2_t[:], in_=a2r[:])

            for i in range(L):
                # ---------- Phase A: zs = dinv * (LN(h) @ W') ----------
                for t in range(TILES):
                    sl = slice(t * 128, (t + 1) * 128)
                    h_t = hpool.tile([128, D], F32, tag="h_t")
                    nc.sync.dma_start(out=h_t[:], in_=h_src[i][sl, :])

                    nsum = spool.tile([128, 1], F32, tag="nsum")
                    nc.vector.tensor_reduce(
                        out=nsum[:], in_=h_t[:], axis=mybir.AxisListType.X,
                        op=mybir.AluOpType.add, negate=True,
                    )
                    negmu = spool.tile([128, 1], F32, tag="negmu")
                    nc.vector.tensor_scalar_mul(negmu[:], nsum[:], 1.0 / D)
                    sqs = spool.tile([128, D], BF16, tag="sqs")
                    ssq = spool.tile([128, 1], F32, tag="ssq")
                    nc.scalar.activation(
                        out=sqs[:], in_=h_t[:],
                        func=mybir.ActivationFunctionType.Square,
                        accum_out=ssq[:],
                    )
                    musq = spool.tile([128, 1], F32, tag="musq")
                    nc.vector.tensor_tensor(
                        out=musq[:], in0=negmu[:], in1=negmu[:],
                        op=mybir.AluOpType.mult,
                    )
                    var = spool.tile([128, 1], F32, tag="var")
                    nc.vector.tensor_scalar(
                        out=var[:], in0=ssq[:], scalar1=1.0 / D, scalar2=musq[:],
                        op0=mybir.AluOpType.mult, op1=mybir.AluOpType.subtract,
                    )
                    std = spool.tile([128, 1], F32, tag="std")
                    nc.scalar.activation(
                        out=std[:], in_=var[:],
                        func=mybir.ActivationFunctionType.Sqrt, bias=eps_col[:],
                    )
                    rstd = spool.tile([128, 1], F32, tag="rstd")
                    nc.vector.reciprocal(rstd[:], std[:])

                    zn = znpool.tile([128, D], BF16, tag="zn")
                    nc.vector.tensor_scalar(
                        out=zn[:], in0=h_t[:], scalar1=negmu[:], scalar2=rstd[:],
                        op0=mybir.AluOpType.add, op1=mybir.AluOpType.mult,
                    )
                    znt = zntpool.tile([128, 256], BF16, tag="znt")
                    for hlf in range(2):
                        pt = pt_pool.tile([128, 128], BF16, space="PSUM", tag="pt")
                        nc.tensor.transpose(
                            out=pt[:], in_=zn[:, hlf * 128:(hlf + 1) * 128],
                            identity=ident_bf[:],
                        )
                        nc.vector.tensor_copy(znt[:, hlf * 128:(hlf + 1) * 128], pt[:])

                    pzw = pzw_pool.tile([128, D], F32, space="PSUM", tag="pzw")
                    nc.tensor.matmul(
                        out=pzw[:], lhsT=znt[:, 0:128], rhs=wp_t[(i, 0)][:],
                        start=True, stop=False,
                    )
                    nc.tensor.matmul(
                        out=pzw[:], lhsT=znt[:, 128:256], rhs=wp_t[(i, 1)][:],
                        start=False, stop=True,
                    )
                    zs_t = zspool.tile([128, D], BF16, tag="zs_t")
                    nc.vector.tensor_scalar_mul(zs_t[:], pzw[:], dinv_t[:, t:t + 1])
                    nc.sync.dma_start(out=zs_loc[i][sl, :], in_=zs_t[:])

                # ---------- AllGather ----------
                nc.gpsimd.collective_compute(
                    "AllGather",
                    mybir.AluOpType.bypass,
                    replica_groups=[list(range(NC))],
                    ins=[zs_loc[i][:]],
                    outs=[zs_full[i][:]],
                )

                # ---------- Phase B: aggregate + GELU + residual ----------
                for t in range(TILES):
                    sl = slice(t * 128, (t + 1) * 128)
                    gix_t = mpool.tile([128, nchunk], I32, tag="gix_t")
                    nc.sync.dma_start(out=gix_t[:], in_=gidx[t])
                    wv_t = mpool.tile([128, nchunk], F32, tag="wv_t")
                    nc.sync.dma_start(out=wv_t[:], in_=wv[t])
                    dstl_t = mpool.tile([128, nchunk], F32, tag="dstl_t")
                    nc.sync.dma_start(out=dstl_t[:], in_=dstl[t])

                    pagg = pagg_pool.tile([128, D], F32, space="PSUM", tag="pagg")
                    nc.tensor.matmul(
                        out=pagg[:], lhsT=a1_t[:, sl], rhs=bpp_t[:, i * D:(i + 1) * D],
                        start=True, stop=False,
                    )
                    nc.tensor.matmul(
                        out=pagg[:], lhsT=a2_t[:, sl], rhs=convb_t[:, i * D:(i + 1) * D],
                        start=False, stop=False,
                    )
                    for j in range(nchunk):
                        msg = msgpool.tile([128, D], BF16, tag="msg")
                        nc.gpsimd.indirect_dma_start(
                            out=msg[:],
                            out_offset=None,
                            in_=zs_full[i][:, :],
                            in_offset=bass.IndirectOffsetOnAxis(
                                ap=gix_t[:, j:j + 1], axis=0
                            ),
                        )
                        selm = selpool.tile([128, 128], BF16, tag="selm")
                        nc.vector.tensor_scalar(
                            out=selm[:], in0=iota_bf[:],
                            scalar1=dstl_t[:, j:j + 1], scalar2=wv_t[:, j:j + 1],
                            op0=mybir.AluOpType.is_equal, op1=mybir.AluOpType.mult,
                        )
                        nc.tensor.matmul(
                            out=pagg[:], lhsT=selm[:], rhs=msg[:],
                            start=False, stop=(j == nchunk - 1),
                        )

                    agg = epool.tile([128, D], F32, tag="agg")
                    nc.vector.tensor_scalar_mul(agg[:], pagg[:], dinv_t[:, t:t + 1])
                    gel = epool.tile([128, D], F32, tag="gel")
                    nc.scalar.activation(
                        out=gel[:], in_=agg[:],
                        func=mybir.ActivationFunctionType.Gelu,
                    )
                    h_t2 = hpool.tile([128, D], F32, tag="h_t2")
                    nc.sync.dma_start(out=h_t2[:], in_=h_src[i][sl, :])
                    hn = epool.tile([128, D], F32, tag="hn")
                    nc.vector.tensor_tensor(
                        out=hn[:], in0=gel[:], in1=h_t2[:], op=mybir.AluOpType.add,
                    )
                    nc.sync.dma_start(out=h_dst[i][sl, :], in_=hn[:])

            # ---------- Final projection: out = h @ post_w + post_b ----------
            for t in range(TILES):
                sl = slice(t * 128, (t + 1) * 128)
                h_t = hpool.tile([128, D], F32, tag="h_t")
                nc.sync.dma_start(out=h_t[:], in_=h_dst[L - 1][sl, :])
                hT = zntpool.tile([128, 256], F32, tag="hT")
                for hlf in range(2):
                    pt = pt_pool.tile([128, 128], F32, space="PSUM", tag="pt")
                    nc.tensor.transpose(
                        out=pt[:], in_=h_t[:, hlf * 128:(hlf + 1) * 128],
                        identity=ident_f32[:],
                    )
                    nc.vector.tensor_copy(hT[:, hlf * 128:(hlf + 1) * 128], pt[:])
                po = pzw_pool.tile([128, D], F32, space="PSUM", tag="po")
                nc.tensor.matmul(
                    out=po[:], lhsT=hT[:, 0:128], rhs=postw_t[0][:],
                    start=True, stop=False,
                )
                nc.tensor.matmul(
                    out=po[:], lhsT=hT[:, 128:256], rhs=postw_t[1][:],
                    start=False, stop=False,
                )
                nc.tensor.matmul(
                    out=po[:], lhsT=ones_row[:], rhs=postb_t[:],
                    start=False, stop=True,
                )
                o_t = epool.tile([128, D], F32, tag="o_t")
                nc.vector.tensor_copy(o_t[:], po[:])
                nc.sync.dma_start(out=out[sl, :], in_=o_t[:])

    nc.finalize()
    return nc


def preprocess(x, edge_index, edge_weight, emb_weight, ln_w, ln_b, conv_w, conv_b,
               post_w, post_b):
    src = np.asarray(edge_index[0], dtype=np.int64)
    dst = np.asarray(edge_index[1], dtype=np.int64)
    w = np.asarray(edge_weight, dtype=np.float64)

    loop = np.arange(N, dtype=np.int64)
    src = np.concatenate([src, loop])
    dst = np.concatenate([dst, loop])
    w = np.concatenate([w, np.ones(N)])

    deg = np.bincount(dst, weights=w, minlength=N)
    dinv = np.where(deg > 0, 1.0 / np.sqrt(np.maximum(deg, 1e-12)), 0.0)

    # per-node bias coefficients: a1 = sum_in(w*dinv_src)  (s = dinv*a1),
    # a2 = 1/dinv  (so that dinv*(a1*b'' + a2*conv_b) = s*b'' + conv_b)
    a1 = np.bincount(dst, weights=w * dinv[src], minlength=N)
    a2 = np.where(dinv > 0, 1.0 / dinv, 0.0)

    # padded-global row id of each source node in the allgathered buffer
    src_row = (src // NS) * NSP + (src % NS)

    shard = dst // NS
    in_maps = []
    percore = []
    max_chunks = 1
    for c in range(NC):
        m = shard == c
        e_src_row = src_row[m]
        e_dst_loc = dst[m] - c * NS
        e_w = w[m]
        order = np.argsort(e_dst_loc, kind="stable")
        e_src_row = e_src_row[order]
        e_dst_loc = e_dst_loc[order]
        e_w = e_w[order]
        t_id = e_dst_loc >> 7
        counts = np.bincount(t_id, minlength=TILES)
        max_chunks = max(max_chunks, int(np.ceil(counts.max() / 128)))
        percore.append((e_src_row, e_dst_loc, e_w, counts))

    K = max_chunks
    for c in range(NC):
        e_src_row, e_dst_loc, e_w, counts = percore[c]
        gidx = np.zeros((TILES, K * 128), dtype=np.int32)
        wv = np.zeros((TILES, K * 128), dtype=np.float32)
        dl = np.zeros((TILES, K * 128), dtype=np.float32)
        offs = np.concatenate([[0], np.cumsum(counts)])
        for t in range(TILES):
            n_t = counts[t]
            s0 = offs[t]
            gidx[t, :n_t] = e_src_row[s0:s0 + n_t]
            wv[t, :n_t] = e_w[s0:s0 + n_t]
            dl[t, :n_t] = e_dst_loc[s0:s0 + n_t] & 127
        # [TILES, K*128] -> [TILES, 128, K] with edge e of chunk j at [t, e, j]
        gidx = gidx.reshape(TILES, K, 128).transpose(0, 2, 1)
        wv = wv.reshape(TILES, K, 128).transpose(0, 2, 1)
        dl = dl.reshape(TILES, K, 128).transpose(0, 2, 1)

        h0 = np.zeros((NSP, D), dtype=np.float32)
        h0[:NS] = np.asarray(emb_weight[c * NS:(c + 1) * NS], dtype=np.float32)

        dinv_loc = np.zeros(NSP, dtype=np.float32)
        dinv_loc[:NS] = dinv[c * NS:(c + 1) * NS]
        a1_loc = np.zeros((1, NSP), dtype=np.float32)
        a1_loc[0, :NS] = a1[c * NS:(c + 1) * NS]
        a2_loc = np.zeros((1, NSP), dtype=np.float32)
        a2_loc[0, :NS] = a2[c * NS:(c + 1) * NS]

        wprime = np.einsum("ld,ldk->ldk", np.asarray(ln_w, np.float64),
                           np.asarray(conv_w, np.float64))
        bprime = np.einsum("ld,ldk->lk", np.asarray(ln_b, np.float64),
                           np.asarray(conv_w, np.float64))

        in_maps.append({
            "h0": h0,
            "gidx": np.ascontiguousarray(gidx),
            "wv": np.ascontiguousarray(wv.astype(np.float32)),
            "dstl": np.ascontiguousarray(dl.astype(np.float32)),
            "dinvc": np.ascontiguousarray(dinv_loc.reshape(TILES, 128).T.astype(np.float32)),
            "a1r": a1_loc,
            "a2r": a2_loc,
            "wp": np.ascontiguousarray(
                wprime.reshape(L, 2, 128, D).astype(NP_BF16)),
            "bpp": bprime.astype(np.float32).reshape(1, L * D),
            "convb": np.asarray(conv_b, dtype=np.float32).reshape(1, L * D),
            "postw": np.ascontiguousarray(
                np.asarray(post_w, np.float32).reshape(2, 128, D)),
            "postb": np.asarray(post_b, np.float32).reshape(1, D),
        })
    return in_maps, K


def kernel(**inputs) -> np.ndarray:
    in_maps, K = preprocess(**inputs)
    if K not in _CACHE:
        _CACHE[K] = build_program(K)
    nc = _CACHE[K]
    res = run_bass_kernel_spmd(nc, in_maps, list(range(NC)))
    outs = [np.asarray(r["out"])[:NS] for r in res.results]
    return np.concatenate(outs, axis=0).astype(np.float32)


if __name__ == "__main__":
    pass
